# revision 1
# baseline (speedup 1.0000x reference)
"""NMI loss (normalized mutual information over soft histograms) on 8 trn2 cores.

Voxel-sharded (per sharding hint): each core processes N/8 = 262144 voxels.
Per 128-voxel group it builds dense I_a / I_b rows (32 Gaussian-window bins)
with DVE (subtract) + ACT (square, exp), normalizes I_a by its row sum, and
accumulates ONE 33x33 Gram matrix on the TensorEngine:
    lhsT = [I_an | 1/S_b]  (bf16), rhs = [I_b | 1]  (bf16)
giving   out[0:32,0:32] = sum I_an*I_b   (N*pab partial)
         out[0:32,32]   = sum I_an       (N*pa  partial)
         out[32,0:32]   = sum I_b/S_b    (N*pb  partial)
The 8 partial 33x33 stats go to the host, which sums them and does the tiny
log-MI reduction (1024 elements) exactly as the reference.

Raw Bass blocks (manual semaphores): the Tile layer's multi-wait sync_info is
rejected by this container's walrus ("Too many sync wait commands"), so the
pipeline below uses standalone wait_ge instructions and depth-2 buffering.
"""

import sys
import numpy as np

sys.path.insert(0, "/opt/trn_rl_repo")

NCORES = 8
P = 128
B = 32                     # bins
S = B + 1                  # slot width (bins + 1 extra column)
NVOX_TOTAL = 128 ** 3      # 2097152
NVOX = NVOX_TOTAL // NCORES
COLS = NVOX // P           # 2048 voxel-columns per core
CHUNK = 64                 # voxel-columns per chunk
NCHUNK = COLS // CHUNK     # 32

# replicate reference's f32 constant computation
_BC = np.linspace(0.0, 1.0, B, dtype=np.float32)
_SIGMA = (np.mean(np.diff(_BC)) * np.float32(0.5)).astype(np.float32)
_PRETERM = (np.float32(1.0) / (np.float32(2.0) * _SIGMA * _SIGMA)).astype(np.float32)

_CACHE = {}


def _build_nc(reps=1):
    from contextlib import ExitStack
    from concourse import bass, mybir

    f32 = mybir.dt.float32
    bf16 = mybir.dt.bfloat16
    AX = mybir.AxisListType
    AF = mybir.ActivationFunctionType

    nc = bass.Bass()
    a_d = nc.dram_tensor("a", [NCHUNK, P, CHUNK], f32, kind="ExternalInput")
    b_d = nc.dram_tensor("b", [NCHUNK, P, CHUNK], f32, kind="ExternalInput")
    iota_d = nc.dram_tensor("iota", [P, B], f32, kind="ExternalInput")
    out_d = nc.dram_tensor("stats", [S, S], f32, kind="ExternalOutput")

    FB = CHUNK * B   # 2048
    FS = CHUNK * S   # 2112

    with ExitStack() as ctx:
        e = ctx.enter_context
        iota_sb = e(nc.sbuf_tensor("iota_sb", [P, B], f32))
        a_t = [e(nc.sbuf_tensor(f"a_t{i}", [P, CHUNK], f32)) for i in range(2)]
        b_t = [e(nc.sbuf_tensor(f"b_t{i}", [P, CHUNK], f32)) for i in range(2)]
        d_a = [e(nc.sbuf_tensor(f"d_a{i}", [P, FB], f32)) for i in range(2)]
        d_b = [e(nc.sbuf_tensor(f"d_b{i}", [P, FB], f32)) for i in range(2)]
        sq_a = [e(nc.sbuf_tensor(f"sq_a{i}", [P, FB], f32)) for i in range(2)]
        sq_b = [e(nc.sbuf_tensor(f"sq_b{i}", [P, FB], f32)) for i in range(2)]
        ia = [e(nc.sbuf_tensor(f"ia{i}", [P, FB], f32)) for i in range(2)]
        ach = [e(nc.sbuf_tensor(f"ach{i}", [P, FS], bf16)) for i in range(2)]
        bch = [e(nc.sbuf_tensor(f"bch{i}", [P, FS], bf16)) for i in range(2)]
        sa = e(nc.sbuf_tensor("sa_sb", [P, CHUNK], f32))
        isa = e(nc.sbuf_tensor("isa_sb", [P, CHUNK], f32))
        sb = e(nc.sbuf_tensor("sb_sb", [P, CHUNK], f32))
        isb = e(nc.sbuf_tensor("isb_sb", [P, CHUNK], f32))
        stats_sb = e(nc.sbuf_tensor("stats_sb", [S, S], f32))
        acc = e(nc.psum_tensor("acc_ps", [S, S], f32))

        s_iota = e(nc.semaphore("s_iota"))
        s_dma_a = e(nc.semaphore("s_dma_a"))
        s_dma_b = e(nc.semaphore("s_dma_b"))
        s_suba = e(nc.semaphore("s_suba"))
        s_subb = e(nc.semaphore("s_subb"))
        s_expa = e(nc.semaphore("s_expa"))
        s_expb = e(nc.semaphore("s_expb"))
        s_ach = e(nc.semaphore("s_ach"))
        s_ones = e(nc.semaphore("s_ones"))
        s_pe = e(nc.semaphore("s_pe"))
        s_done = e(nc.semaphore("s_done"))
        s_out = e(nc.semaphore("s_out"))
        block = e(nc.Block())

        def r3(ap, inner):
            return ap[:, :].rearrange("p (v i) -> p v i", i=inner)

        iota_bc = (
            iota_sb[:, :]
            .rearrange("p (o i) -> p o i", o=1)
            .broadcast_to([P, CHUNK, B])
        )

        G = reps * NCHUNK

        @block.sync
        def _(sync):
            sync.dma_start(iota_sb[:, :], iota_d[:, :]).then_inc(s_iota, 16)
            for g in range(G):
                c = g % NCHUNK
                if g >= 2:
                    sync.wait_ge(s_suba, g - 1)
                sync.dma_start(a_t[g % 2][:, :], a_d[c]).then_inc(s_dma_a, 16)
                if g >= 2:
                    sync.wait_ge(s_subb, g - 1)
                sync.dma_start(b_t[g % 2][:, :], b_d[c]).then_inc(s_dma_b, 16)

        @block.gpsimd
        def _(gpsimd):
            # ones in the B-side extra slot, once per buffer
            for k in range(2):
                gpsimd.memset(r3(bch[k], S)[:, :, B : B + 1], 1.0).then_inc(s_ones, 1)
            gpsimd.wait_ge(s_done, 1)
            gpsimd.dma_start(out_d[:, :], stats_sb[:, :]).then_inc(s_out, 16)
            gpsimd.wait_ge(s_out, 16)

        @block.vector
        def _(vector):
            vector.wait_ge(s_iota, 16)
            for c in range(G):
                k = c % 2
                vector.wait_ge(s_dma_a, 16 * (c + 1))
                vector.tensor_sub(
                    r3(d_a[k], B),
                    a_t[k][:, :].broadcast_to([P, CHUNK, B]),
                    iota_bc,
                ).then_inc(s_suba, 1)
                vector.wait_ge(s_dma_b, 16 * (c + 1))
                vector.tensor_sub(
                    r3(d_b[k], B),
                    b_t[k][:, :].broadcast_to([P, CHUNK, B]),
                    iota_bc,
                ).then_inc(s_subb, 1)

                vector.wait_ge(s_expa, c + 1)
                vector.reduce_sum(sa[:, :], r3(ia[k], B), axis=AX.X)
                vector.reciprocal(isa[:, :], sa[:, :])
                vector.wait_ge(s_expb, c + 1)
                vector.reduce_sum(sb[:, :], r3(bch[k], S)[:, :, 0:B], axis=AX.X)
                vector.reciprocal(isb[:, :], sb[:, :])

                if c >= 2:
                    vector.wait_ge(s_pe, c - 1)
                vector.tensor_mul(
                    r3(ach[k], S)[:, :, 0:B],
                    r3(ia[k], B),
                    isa[:, :]
                    .rearrange("p (v o) -> p v o", o=1)
                    .broadcast_to([P, CHUNK, B]),
                )
                vector.tensor_copy(
                    r3(ach[k], S)[:, :, B : B + 1],
                    isb[:, :].rearrange("p (v o) -> p v o", o=1),
                ).then_inc(s_ach, 1)

            vector.wait_ge(s_pe, G)
            vector.tensor_copy(stats_sb[:, :], acc[:, :]).then_inc(s_done, 1)

        @block.scalar
        def _(scalar):
            for c in range(G):
                k = c % 2
                scalar.wait_ge(s_suba, c + 1)
                scalar.activation(sq_a[k][:, :], d_a[k][:, :], AF.Square)
                if c >= 2:
                    scalar.wait_ge(s_ach, c - 1)
                scalar.activation(
                    ia[k][:, :], sq_a[k][:, :], AF.Exp, scale=float(-_PRETERM)
                ).then_inc(s_expa, 1)

                scalar.wait_ge(s_subb, c + 1)
                scalar.activation(sq_b[k][:, :], d_b[k][:, :], AF.Square)
                if c >= 2:
                    scalar.wait_ge(s_pe, c - 1)
                elif c == 0:
                    scalar.wait_ge(s_ones, 2)
                scalar.activation(
                    r3(bch[k], S)[:, :, 0:B],
                    r3(sq_b[k], B),
                    AF.Exp,
                    scale=float(-_PRETERM),
                ).then_inc(s_expb, 1)

        @block.tensor
        def _(tensor):
            for c in range(G):
                k = c % 2
                tensor.wait_ge(s_ach, c + 1)
                tensor.wait_ge(s_expb, c + 1)
                for v in range(CHUNK):
                    first = c % NCHUNK == 0 and v == 0
                    last = c % NCHUNK == NCHUNK - 1 and v == CHUNK - 1
                    mm = tensor.matmul(
                        acc[:, :],
                        ach[k][:, v * S : (v + 1) * S],
                        bch[k][:, v * S : (v + 1) * S],
                        start=first,
                        stop=last,
                    )
                    if v == CHUNK - 1:
                        mm.then_inc(s_pe, 1)

    return nc


def _get_nc():
    if "nc" not in _CACHE:
        _CACHE["nc"] = _build_nc()
    return _CACHE["nc"]


def run_device(a_flat, b_flat, trace=False):
    """Run the per-core bass kernel on 8 cores; returns (stats_sum, bass_results)."""
    from concourse.bass_utils import run_bass_kernel_spmd

    nc = _get_nc()
    iota_tile = np.tile(_BC[None, :], (P, 1)).astype(np.float32)

    def shard(x, i):
        sl = x[i * NVOX : (i + 1) * NVOX].reshape(P, NCHUNK, CHUNK)
        return np.ascontiguousarray(sl.transpose(1, 0, 2))

    in_maps = []
    for i in range(NCORES):
        in_maps.append(
            {"a": shard(a_flat, i), "b": shard(b_flat, i), "iota": iota_tile}
        )
    kw = {}
    if trace:
        kw.update(trace=True, trace_cores=[0])
    res = run_bass_kernel_spmd(nc, in_maps, list(range(NCORES)), **kw)
    stats = np.zeros((S, S), np.float64)
    for r in res.results:
        stats += np.asarray(r["stats"], np.float64)
    return stats, res


def finish(stats):
    n = float(NVOX_TOTAL)
    pab = stats[0:B, 0:B] / n
    pa = stats[0:B, B] / n
    pb = stats[B, 0:B] / n
    eps = 1.4e-45
    papb = np.outer(pa, pb) + eps
    mi = np.sum(pab * np.log(pab / papb + eps))
    return np.array([-mi], dtype=np.float32)


def kernel(actual, target):
    a = np.clip(np.asarray(actual, np.float32).reshape(-1), 0.0, 1.0)
    b = np.clip(np.asarray(target, np.float32).reshape(-1), 0.0, 1.0)
    stats, _ = run_device(a, b)
    return finish(stats)



# revision 2
# speedup vs baseline: 1.0793x; 1.0793x over previous
"""NMI loss v2: engine-balanced Gaussian soft-histogram on 8 trn2 cores.

Per core: N = 262144 voxels as [128 part, 2048 cols], chunks of VC=512 cols,
double-buffered bin-major I-tile pairs [128, 33, VC] (u16 raw; fp16 y view,
bf16 I view, in-place strided ACT exp).

y = (31*a - k)^2 per bin k:
  a-side (32 bins, DVE): TS d=31*ah-k (fp16 4x) -> TT d*d (2x)
         -> TT += 0.5*ln(S_a) bcast (2x)   [folds 1/S_a into the exp]
  b-side: bins [0,BSPLIT) DVE, bins [BSPLIT,32) ACT Square(31*bh - k).
  exp: ACT Exp(scale=-2) in place fp16->bf16.
S(t) analytic: 1.2533141*(1+0.01441324*cos(2pi t)) - e^{-2(t+1)^2}
  - e^{-2(32-t)^2}; cos via range-reduced ACT Sin (fp16 magic rounding).
Gram: lhsT=[I_an | inv_sb], rhs=[I_b | 1]; 2048 strided 33-col matmuls into
one [33,33] f32 PSUM; host sums 8 core stats + log-MI.
Inputs shipped fp16 (input quantization = position jitter, averages out).
"""

import sys
import numpy as np

sys.path.insert(0, "/opt/trn_rl_repo")

NCORES = 8
P = 128
B = 32
S = B + 1
NVOX_TOTAL = 128 ** 3
NVOX = NVOX_TOTAL // NCORES
COLS = NVOX // P            # 2048
VC = 512
NCH = COLS // VC            # 4
BSPLIT = 20

_CACHE = {}

PI = float(np.pi)
C_P0 = 1.2533141373155003
C_P1 = 0.014413237061177604 * C_P0


def _build_nc():
    from contextlib import ExitStack
    from concourse import bass, mybir

    f32 = mybir.dt.float32
    fp16 = mybir.dt.float16
    bf16 = mybir.dt.bfloat16
    u16 = mybir.dt.uint16
    AF = mybir.ActivationFunctionType
    A = mybir.AluOpType

    nc = bass.Bass()
    a_d = nc.dram_tensor("a", [P, COLS], fp16, kind="ExternalInput")
    b_d = nc.dram_tensor("b", [P, COLS], fp16, kind="ExternalInput")
    out_d = nc.dram_tensor("stats", [S, S], f32, kind="ExternalOutput")

    with ExitStack() as ctx:
        e = ctx.enter_context
        ah = e(nc.sbuf_tensor("ah", [P, COLS], fp16))
        bh = e(nc.sbuf_tensor("bh", [P, COLS], fp16))
        ia = [e(nc.sbuf_tensor(f"ia{i}", [P, S * VC], u16)) for i in range(2)]
        ib = [e(nc.sbuf_tensor(f"ib{i}", [P, S * VC], u16)) for i in range(2)]
        th = e(nc.sbuf_tensor("th", [P, COLS], fp16))
        tf = e(nc.sbuf_tensor("tf", [P, COLS], f32))
        cs = e(nc.sbuf_tensor("cs", [P, COLS], f32))
        ssum = e(nc.sbuf_tensor("ssum", [P, COLS], f32))
        mya = e(nc.sbuf_tensor("mya", [P, COLS], fp16))
        invb = e(nc.sbuf_tensor("invb", [P, COLS], bf16))
        bias_pi2 = e(nc.sbuf_tensor("bias_pi2", [P, 1], f32))
        bias_m32 = e(nc.sbuf_tensor("bias_m32", [P, 1], f32))
        bias_k = [
            e(nc.sbuf_tensor(f"bias_k{kb}", [P, 1], f32))
            for kb in range(BSPLIT, B)
        ]
        stats_sb = e(nc.sbuf_tensor("stats_sb", [S, S], f32))
        acc = e(nc.psum_tensor("acc_ps", [S, S], f32))

        s_in = e(nc.semaphore("s_in"))
        s_pre = e(nc.semaphore("s_pre"))
        s_va = e(nc.semaphore("s_va"))
        s_sa = e(nc.semaphore("s_sa"))
        s_bld_a = e(nc.semaphore("s_bld_a"))
        s_bld_b = e(nc.semaphore("s_bld_b"))
        s_exp_a = e(nc.semaphore("s_exp_a"))
        s_exp_b = e(nc.semaphore("s_exp_b"))
        s_pe = e(nc.semaphore("s_pe"))
        s_done = e(nc.semaphore("s_done"))
        s_out = e(nc.semaphore("s_out"))
        block = e(nc.Block())

        def i3(buf, dt):
            return buf[:, :].rearrange("p (s n) -> p s n", s=S).bitcast(dt)

        @block.sync
        def _(sync):
            sync.dma_start(ah[:, :], a_d[:, :]).then_inc(s_in, 16)
            sync.dma_start(bh[:, :], b_d[:, :]).then_inc(s_in, 16)

        @block.gpsimd
        def _(g):
            g.memset(bias_pi2[:, :], PI / 2)
            g.memset(bias_m32[:, :], -32.0)
            for i, kb in enumerate(range(BSPLIT, B)):
                g.memset(bias_k[i][:, :], -float(kb))
            for i in range(2):
                g.memset(i3(ib[i], bf16)[:, B, :], 1.0).then_inc(s_pre, 1)
            g.wait_ge(s_done, 1)
            g.dma_start(out_d[:, :], stats_sb[:, :]).then_inc(s_out, 16)
            g.wait_ge(s_out, 16)

        @block.vector
        def _(v):
            v.wait_ge(s_in, 32)
            # ---- S-phase: side a (counters: s_va, s_sa) ----
            for side, x in enumerate([ah, bh]):
                base_v = side * 3   # V-steps that inc per side: 3
                base_s = side * 6
                v.tensor_scalar(
                    out=th[:, :], in0=x[:, :], scalar1=31.0, scalar2=1536.0,
                    op0=A.mult, op1=A.add,
                )
                v.tensor_scalar(
                    out=tf[:, :], in0=x[:, :], scalar1=31.0, scalar2=None,
                    op0=A.mult,
                )
                v.tensor_copy(cs[:, :], th[:, :])
                v.scalar_tensor_tensor(
                    out=cs[:, :], in0=cs[:, :], scalar=-1536.0, in1=tf[:, :],
                    op0=A.add, op1=A.subtract,
                ).then_inc(s_va, 1)                       # s_va = base_v+1
                v.wait_ge(s_sa, base_s + 1)               # Sin done
                v.tensor_scalar(
                    out=ssum[:, :], in0=cs[:, :], scalar1=C_P1, scalar2=C_P0,
                    op0=A.mult, op1=A.add,
                )
                v.wait_ge(s_sa, base_s + 3)               # Exp(tail1) done
                v.tensor_tensor(
                    ssum[:, :], ssum[:, :], tf[:, :], op=A.subtract
                ).then_inc(s_va, 1)                       # base_v+2
                v.wait_ge(s_sa, base_s + 5)               # Exp(tail2) done
                v.tensor_tensor(
                    ssum[:, :], ssum[:, :], tf[:, :], op=A.subtract
                ).then_inc(s_va, 1)                       # base_v+3
                if side == 0:
                    v.wait_ge(s_sa, 6)                    # Ln done
                    v.tensor_scalar(
                        out=mya[:, :], in0=cs[:, :], scalar1=0.5,
                        scalar2=None, op0=A.mult,
                    )
                else:
                    v.reciprocal(cs[:, :], ssum[:, :])
                    v.tensor_copy(invb[:, :], cs[:, :])

            # ---- main chunk loop ----
            for c in range(NCH):
                k = c % 2
                iaf = i3(ia[k], fp16)
                ibf = i3(ib[k], fp16)
                asl = ah[:, c * VC : (c + 1) * VC]
                bsl = bh[:, c * VC : (c + 1) * VC]
                if c >= 2:
                    v.wait_ge(s_pe, c - 1)
                for kb in range(B):
                    v.tensor_scalar(
                        out=iaf[:, kb, :], in0=asl, scalar1=31.0,
                        scalar2=-float(kb), op0=A.mult, op1=A.add,
                    )
                da = iaf[:, 0:B, :]
                v.tensor_tensor(da, da, da, op=A.mult)
                msl = (
                    mya[:, c * VC : (c + 1) * VC]
                    .rearrange("p (o n) -> p o n", o=1)
                    .broadcast_to([P, B, VC])
                )
                v.tensor_tensor(da, da, msl, op=A.add)
                v.tensor_copy(
                    i3(ia[k], bf16)[:, B, :], invb[:, c * VC : (c + 1) * VC]
                ).then_inc(s_bld_a, 1)
                for kb in range(BSPLIT):
                    v.tensor_scalar(
                        out=ibf[:, kb, :], in0=bsl, scalar1=31.0,
                        scalar2=-float(kb), op0=A.mult, op1=A.add,
                    )
                db = ibf[:, 0:BSPLIT, :]
                v.tensor_tensor(db, db, db, op=A.mult).then_inc(s_bld_b, 1)

            v.wait_ge(s_pe, NCH)
            v.tensor_copy(stats_sb[:, :], acc[:, :]).then_inc(s_done, 1)

        @block.scalar
        def _(sc):
            sc.wait_ge(s_in, 32)
            sc.wait_ge(s_pre, 2)
            # ---- S-phase ACT ----
            for side, x in enumerate([ah, bh]):
                base_v = side * 3
                sc.wait_ge(s_va, base_v + 1)
                sc.activation(
                    cs[:, :], cs[:, :], AF.Sin, scale=-2.0 * PI,
                    bias=bias_pi2[:, :],
                ).then_inc(s_sa, 1)
                sc.activation(
                    tf[:, :], x[:, :], AF.Square, scale=31.0, bias=1.0
                ).then_inc(s_sa, 1)
                sc.activation(
                    tf[:, :], tf[:, :], AF.Exp, scale=-2.0
                ).then_inc(s_sa, 1)
                sc.wait_ge(s_va, base_v + 2)
                sc.activation(
                    tf[:, :], x[:, :], AF.Square, scale=31.0,
                    bias=bias_m32[:, :],
                ).then_inc(s_sa, 1)
                sc.activation(
                    tf[:, :], tf[:, :], AF.Exp, scale=-2.0
                ).then_inc(s_sa, 1)
                if side == 0:
                    sc.wait_ge(s_va, 3)
                    sc.activation(
                        cs[:, :], ssum[:, :], AF.Ln, scale=1.0
                    ).then_inc(s_sa, 1)
                else:
                    sc.activation(
                        tf[:, 0:1], tf[:, 0:1], AF.Exp, scale=1.0
                    ).then_inc(s_sa, 1)

            # ---- main loop ACT ----
            for c in range(NCH):
                k = c % 2
                iaf = i3(ia[k], fp16)
                ibf = i3(ib[k], fp16)
                bsl = bh[:, c * VC : (c + 1) * VC]
                if c >= 2:
                    sc.wait_ge(s_pe, c - 1)
                for i, kb in enumerate(range(BSPLIT, B)):
                    sc.activation(
                        ibf[:, kb, :], bsl, AF.Square, scale=31.0,
                        bias=bias_k[i][:, :],
                    )
                sc.wait_ge(s_bld_a, c + 1)
                H = VC // 2
                for h in range(2):
                    sc.activation(
                        i3(ia[k], bf16)[:, 0:B, h * H : (h + 1) * H],
                        iaf[:, 0:B, h * H : (h + 1) * H], AF.Exp,
                        scale=-2.0,
                    ).then_inc(s_exp_a, 1)
                sc.wait_ge(s_bld_b, c + 1)
                for h in range(2):
                    sc.activation(
                        i3(ib[k], bf16)[:, 0:B, h * H : (h + 1) * H],
                        ibf[:, 0:B, h * H : (h + 1) * H], AF.Exp,
                        scale=-2.0,
                    ).then_inc(s_exp_b, 1)

        @block.tensor
        def _(t):
            for c in range(NCH):
                k = c % 2
                ia_b = i3(ia[k], bf16)
                ib_b = i3(ib[k], bf16)
                for h in range(2):
                    t.wait_ge(s_exp_a, 2 * c + h + 1)
                    t.wait_ge(s_exp_b, 2 * c + h + 1)
                    for vv in range(h * VC // 2, (h + 1) * VC // 2):
                        first = c == 0 and vv == 0
                        last = c == NCH - 1 and vv == VC - 1
                        mm = t.matmul(
                            acc[:, :], ia_b[:, :, vv], ib_b[:, :, vv],
                            start=first, stop=last,
                        )
                        if vv == VC - 1:
                            mm.then_inc(s_pe, 1)

    return nc


def _get_nc():
    if "nc" not in _CACHE:
        _CACHE["nc"] = _build_nc()
    return _CACHE["nc"]


def run_device(a_flat, b_flat, trace=False):
    from concourse.bass_utils import run_bass_kernel_spmd

    nc = _get_nc()

    def shard(x, i):
        sl = x[i * NVOX : (i + 1) * NVOX].reshape(P, COLS)
        return np.ascontiguousarray(sl).astype(np.float16)

    in_maps = [
        {"a": shard(a_flat, i), "b": shard(b_flat, i)} for i in range(NCORES)
    ]
    kw = {}
    if trace:
        kw.update(trace=True, trace_cores=[0])
    res = run_bass_kernel_spmd(nc, in_maps, list(range(NCORES)), **kw)
    stats = np.zeros((S, S), np.float64)
    for r in res.results:
        stats += np.asarray(r["stats"], np.float64)
    return stats, res


def finish(stats):
    n = float(NVOX_TOTAL)
    pab = stats[0:B, 0:B] / n
    pa = stats[0:B, B] / n
    pb = stats[B, 0:B] / n
    eps = 1.4e-45
    papb = np.outer(pa, pb) + eps
    mi = np.sum(pab * np.log(pab / papb + eps))
    return np.array([-mi], dtype=np.float32)


def kernel(actual, target):
    a = np.clip(np.asarray(actual, np.float32).reshape(-1), 0.0, 1.0)
    b = np.clip(np.asarray(target, np.float32).reshape(-1), 0.0, 1.0)
    stats, _ = run_device(a, b)
    return finish(stats)
